# revision 37
# baseline (speedup 1.0000x reference)
"""Trainium2 Bass kernel for a DeepSeek-style MoE block (full-I/O contract).

Strategy (8 NeuronCores):
  - Expert-parallel: E=16 routed experts, 2 per core. Host computes the gate
    (softmax + top-4) in numpy, gathers each expert's tokens, and ships
    transposed token blocks per core. Experts are ranked by token count:
    ranks 0-7 go to slot 0 (capacity C0), ranks 8-15 to slot 1 (C1 <= C0),
    so padding waste tracks the actual load distribution.
  - Routed path runs in fp8 e4m3 with DoubleRow matmuls (256-deep
    contraction per instruction, 2x PE throughput). Weights are scaled
    into fp8 range on the host (w_gate/w_down by 64, w_up by 4); the
    gate dequant rides the SiLU activation's input scale and the rest is
    folded into the per-token routing weights at PSUM eviction. The
    shared expert stays in bf16: it carries ~98% of the output norm,
    while the routed path (~22%) tolerates fp8 noise.
  - Shared expert is tensor-parallel along its intermediate dim Fs=2816:
    each core owns a 352-wide slice (zero-padded to 384 = 3*128).
  - Host scatter-adds the routed partials and sums the shared partials.
"""

import math
from contextlib import ExitStack

import ml_dtypes
import numpy as np

T = 2048
H = 2048
E = 16
TOP_K = 4
F = 1408
FS = 2816
N_CORES = 8
EPC = E // N_CORES  # experts per core = 2
KH = H // 128  # 16 contraction chunks over H
KH2 = KH // 2  # 8 DoubleRow pairs
NF = F // 128  # 11 F tiles
NF2 = NF // 2  # 5 DoubleRow pairs (+1 leftover chunk)
NH = H // 512  # 4 output H tiles
FSS = FS // N_CORES  # 352 shared slice per core
FSP = 384  # padded to 3*128
NFS = FSP // 128  # 3
NT = T // 512  # 4

S_W = 64.0  # fp8 scale for w_gate / w_down
S_WU = 4.0  # fp8 scale for w_up (rides through silu(g)*u into the a tiles)

BF16 = ml_dtypes.bfloat16
F8 = ml_dtypes.float8_e4m3

_BUILD_CACHE: dict[tuple, object] = {}
last_exec_time_ns = None


def _routing(x: np.ndarray, gate_weight: np.ndarray):
    """Replicates the reference gate: fp32 logits, softmax, top-4 (ties ->
    lower expert index, matching jax.lax.top_k), no renorm."""
    logits = x.astype(np.float32) @ gate_weight.astype(np.float32).T
    z = logits - logits.max(axis=1, keepdims=True)
    p = np.exp(z)
    p /= p.sum(axis=1, keepdims=True)
    top_idx = np.argsort(-p, axis=1, kind="stable")[:, :TOP_K]
    top_vals = np.take_along_axis(p, top_idx, axis=1).astype(np.float32)
    return top_idx, top_vals


def _chunks(C):
    n = max(1, math.ceil(C / 512))
    while C % n:
        n += 1
    return C // n


def _build(caps: tuple):
    """Build + compile the SPMD one-core Bass graph for slot capacities."""
    key = tuple(caps)
    if key in _BUILD_CACHE:
        return _BUILD_CACHE[key]

    import concourse.bass as bass  # noqa: F401
    from concourse import bacc, mybir, tile

    bf = mybir.dt.bfloat16
    f8 = mybir.dt.float8e4
    f32 = mybir.dt.float32
    Silu = mybir.ActivationFunctionType.Silu
    DR = mybir.MatmulPerfMode.DoubleRow

    off = [0, caps[0]]  # column offsets into rout / wts

    nc = bacc.Bacc(None, target_bir_lowering=False)

    xg_ds = [
        nc.dram_tensor(f"xg{e}", [128, KH, caps[e]], f8, kind="ExternalInput")
        for e in range(EPC)
    ]
    wg_d = nc.dram_tensor("wg", [EPC, NF, 128, KH, 128], f8, kind="ExternalInput")
    wu_d = nc.dram_tensor("wu", [EPC, NF, 128, KH, 128], f8, kind="ExternalInput")
    wd_d = nc.dram_tensor("wd", [EPC, NH, 128, NF, 512], f8, kind="ExternalInput")
    xt_d = nc.dram_tensor("xt", [NT, 128, KH, 512], bf, kind="ExternalInput")
    wsg_d = nc.dram_tensor("wsg", [NFS, 128, KH, 128], bf, kind="ExternalInput")
    wsu_d = nc.dram_tensor("wsu", [NFS, 128, KH, 128], bf, kind="ExternalInput")
    wsd_d = nc.dram_tensor("wsd", [NFS, 128, NH, 512], bf, kind="ExternalInput")
    wts_d = nc.dram_tensor("wts", [128, sum(caps)], f32, kind="ExternalInput")
    # outputs are block-contiguous per eviction tile (one DMA descriptor
    # each); the host reassembles token/H-major layouts
    rout_ds = [
        nc.dram_tensor(
            f"rout{e}", [NH, caps[e] // _chunks(caps[e]), 128, 4, _chunks(caps[e])],
            bf, kind="ExternalOutput",
        )
        for e in range(EPC)
    ]
    sout = nc.dram_tensor("sout", [NT, 4, 128, NH, 512], bf, kind="ExternalOutput")

    with tile.TileContext(nc) as tc, ExitStack() as ctx:
        const = ctx.enter_context(tc.tile_pool(name="const", bufs=1))
        bias0 = const.tile([128, 1], f32)
        nc.vector.memset(bias0[:], 0.0)
        # routing weights broadcast across partitions: wts_t[p, col] = w[col]
        wts_t = const.tile([128, sum(caps)], f32)
        nc.sync.dma_start(wts_t[:], wts_d[:])


        xt_pool = ctx.enter_context(tc.tile_pool(name="xtp", bufs=4))
        wgu_pool = ctx.enter_context(tc.tile_pool(name="wgu", bufs=4))
        as_pool = ctx.enter_context(tc.tile_pool(name="asp", bufs=2))
        wsd_pool = ctx.enter_context(tc.tile_pool(name="wsdp", bufs=1))
        ev_pool = ctx.enter_context(tc.tile_pool(name="evp", bufs=4))
        sg_pool = ctx.enter_context(tc.tile_pool(name="sgp", bufs=4))
        psum_gu = ctx.enter_context(tc.tile_pool(name="pgu", bufs=2, space="PSUM"))
        psum_d = ctx.enter_context(tc.tile_pool(name="pdp", bufs=4, space="PSUM"))

        # HAM warmup: keep the PE busy during the initial DMA wait so the
        # clock-gate is at 8/8 when the first real matmuls arrive
        warm = const.tile([128, 512], bf, name="warm")
        nc.vector.memset(warm[:], 0.0)
        warmout = const.tile([128, 1], f32, name="warmout")
        wpsum = psum_gu.tile([128, 512], f32, tag="pg", name="warmp")
        for _ in range(30):
            nc.tensor.matmul(wpsum, warm[:, :128], warm[:], start=True, stop=True)
        nc.vector.tensor_copy(out=warmout[:], in_=wpsum[:, :1])

        # ---- routed experts (2 per core, fp8 DoubleRow, C0 >= C1) ----
        with (
            tc.tile_pool(name="xgp", bufs=1) as xg_pool,
            tc.tile_pool(name="wdp", bufs=2) as wd_pool,
            tc.tile_pool(name="atp", bufs=1) as a_pool,
        ):
            for e in range(EPC):
                C = caps[e]
                cw = _chunks(C)
                nch = C // cw
                xg_t = xg_pool.tile([128, KH, C], f8, tag=f"xg{e}", name=f"xg{e}")
                for q in range(4):
                    nc.gpsimd.dma_start(
                        xg_t[:, q * 4:(q + 1) * 4], xg_ds[e][:, q * 4:(q + 1) * 4]
                    )
                aT = a_pool.tile([128, NF, C], f8, tag=f"aT{e}", name=f"aT{e}")
                for f in range(NF):
                    wg_t = wgu_pool.tile([128, KH, 128], f8, tag="wg")
                    nc.gpsimd.dma_start(wg_t[:], wg_d[e, f])
                    wu_t = wgu_pool.tile([128, KH, 128], f8, tag="wu")
                    nc.scalar.dma_start(wu_t[:], wu_d[e, f])
                    for j in range(nch):
                        csl = slice(j * cw, (j + 1) * cw)
                        pg = psum_gu.tile([128, 512], f32, tag="pg", name="pg")[:, :cw]
                        pu = psum_gu.tile([128, 512], f32, tag="pu", name="pu")[:, :cw]
                        for k in range(KH2):
                            nc.tensor.matmul(
                                pg, wg_t[:, 2 * k:2 * k + 2],
                                xg_t[:, 2 * k:2 * k + 2, csl],
                                start=(k == 0), stop=(k == KH2 - 1),
                                perf_mode=DR,
                            )
                        for k in range(KH2):
                            nc.tensor.matmul(
                                pu, wu_t[:, 2 * k:2 * k + 2],
                                xg_t[:, 2 * k:2 * k + 2, csl],
                                start=(k == 0), stop=(k == KH2 - 1),
                                perf_mode=DR,
                            )
                        sg = sg_pool.tile([128, 512], f32, tag="sg", name="sg")[:, :cw]
                        # dequant w_gate's 64x scale ahead of the nonlinearity
                        nc.scalar.activation(
                            sg, pg, Silu, bias=bias0[:], scale=1.0 / S_W
                        )
                        # a tile keeps w_up's 4x scale; folded into wts below
                        nc.vector.tensor_mul(aT[:, f, csl], sg, pu)
                # down-proj with w_down stationary and token columns moving:
                # cost scales with C, not ceil(C/128)*512
                for h in range(NH):
                    wd_t = wd_pool.tile([128, NF, 512], f8, tag="wd")
                    nc.gpsimd.dma_start(wd_t[:, :6], wd_d[e, h, :, :6])
                    nc.gpsimd.dma_start(wd_t[:, 6:], wd_d[e, h, :, 6:])
                    for j in range(nch):
                        csl = slice(j * cw, (j + 1) * cw)
                        ob = ev_pool.tile([128, 4, 512], bf, tag="ob", name="ob")[
                            :, :, :cw
                        ]
                        for hs in range(4):
                            hsl = slice(hs * 128, (hs + 1) * 128)
                            pd = psum_d.tile([128, 512], f32, tag="pd", name="pd")[
                                :, :cw
                            ]
                            for f2 in range(NF2):
                                nc.tensor.matmul(
                                    pd, wd_t[:, 2 * f2:2 * f2 + 2, hsl],
                                    aT[:, 2 * f2:2 * f2 + 2, csl],
                                    start=(f2 == 0), stop=False,
                                    perf_mode=DR,
                                )
                            nc.tensor.matmul(
                                pd, wd_t[:, NF - 1, hsl], aT[:, NF - 1, csl],
                                start=False, stop=True,
                            )
                            nc.vector.tensor_mul(
                                ob[:, hs], pd[:], wts_t[:, off[e] + j * cw:
                                                        off[e] + j * cw + cw]
                            )
                        nc.scalar.dma_start(rout_ds[e][h, j], ob[:])


        # ---- shared expert (Fs tensor-parallel slice, bf16) ----
        wsd_ts = []
        for j in range(NT):
            xt_t = xt_pool.tile([128, KH, 512], bf, tag="xt", name="xt")
            for q in range(4):
                nc.sync.dma_start(
                    xt_t[:, q * 4:(q + 1) * 4], xt_d[j, :, q * 4:(q + 1) * 4]
                )
            as_j = as_pool.tile([128, NFS, 512], bf, tag="asj", name="asj")
            for f in range(NFS):
                wsg_t = wgu_pool.tile([128, KH, 128], bf, tag="wg")
                wsu_t = wgu_pool.tile([128, KH, 128], bf, tag="wu")
                nc.sync.dma_start(wsg_t[:, :8], wsg_d[f, :, :8])
                nc.sync.dma_start(wsg_t[:, 8:], wsg_d[f, :, 8:])
                nc.sync.dma_start(wsu_t[:, :8], wsu_d[f, :, :8])
                nc.sync.dma_start(wsu_t[:, 8:], wsu_d[f, :, 8:])
                pg = psum_gu.tile([128, 512], f32, tag="pg", name="pg")
                pu = psum_gu.tile([128, 512], f32, tag="pu", name="pu")
                for k in range(KH):
                    nc.tensor.matmul(
                        pg, wsg_t[:, k], xt_t[:, k],
                        start=(k == 0), stop=(k == KH - 1),
                    )
                for k in range(KH):
                    nc.tensor.matmul(
                        pu, wsu_t[:, k], xt_t[:, k],
                        start=(k == 0), stop=(k == KH - 1),
                    )
                sg = sg_pool.tile([128, 512], f32, tag="sg", name="sg")
                nc.scalar.activation(sg, pg, Silu, bias=bias0[:])
                nc.vector.tensor_mul(as_j[:, f], sg, pu)
            if not wsd_ts:
                for cc in range(NFS):
                    w = wsd_pool.tile(
                        [128, NH, 512], bf, tag=f"wsd{cc}", name=f"wsd{cc}"
                    )
                    nc.sync.dma_start(w[:, :2], wsd_d[cc, :, :2])
                    nc.sync.dma_start(w[:, 2:], wsd_d[cc, :, 2:])
                    wsd_ts.append(w)
            # down-proj for this T-chunk right away: spreads sout writes
            for tl in range(4):
                ob = ev_pool.tile([128, NH, 512], bf, tag="ob")
                for h in range(NH):
                    pd = psum_d.tile([128, 512], f32, tag="pd")
                    for fo in range(NFS):
                        nc.tensor.matmul(
                            pd, as_j[:, fo, tl * 128:(tl + 1) * 128],
                            wsd_ts[fo][:, h],
                            start=(fo == 0), stop=(fo == NFS - 1),
                        )
                    nc.vector.tensor_copy(out=ob[:, h], in_=pd[:])
                if j == NT - 1 and tl == 3:
                    # drain the final eviction in narrow pieces on all
                    # three queues to shorten the kernel tail
                    engs = [nc.scalar, nc.gpsimd, nc.sync, nc.scalar]
                    for h in range(NH):
                        engs[h].dma_start(sout[j, tl, :, h], ob[:, h])
                else:
                    eng = nc.scalar if tl % 2 else nc.gpsimd
                    eng.dma_start(sout[j, tl], ob[:])

    nc.compile()
    _BUILD_CACHE[key] = nc
    return nc


def kernel(**inputs: np.ndarray) -> np.ndarray:
    global last_exec_time_ns
    from concourse.bass_utils import run_bass_kernel_spmd

    hs = inputs["hidden_states"]
    x = np.ascontiguousarray(hs.reshape(T, H), dtype=np.float32)
    top_idx, top_vals = _routing(x, inputs["gate_weight"])

    # per-expert token lists (ascending token order)
    rows_per_e = []
    for e in range(E):
        rows, kpos = np.nonzero(top_idx == e)
        rows_per_e.append((rows, top_vals[rows, kpos]))
    counts = np.array([len(r) for r, _ in rows_per_e])
    # rank experts by load: ranks 0..7 -> slot 0 of cores 0..7 (big slots),
    # ranks 8..15 -> slot 1 of cores 7..0 (small slots)
    order = np.argsort(-counts, kind="stable")
    slot_expert = np.empty((N_CORES, EPC), np.int64)
    for i in range(N_CORES):
        slot_expert[i, 0] = order[i]
        slot_expert[i, 1] = order[E - 1 - i]
    cap = lambda n: max(128, ((n + 15) // 16) * 16)
    caps = (
        cap(int(counts[slot_expert[:, 0]].max())),
        cap(int(counts[slot_expert[:, 1]].max())),
    )

    nc = _build(caps)

    xb = x.astype(BF16)
    x8 = x.astype(F8)
    # xt chunks [NT, 128, KH, 512]: xt[j, p, k, t'] = x[j*512+t', k*128+p]
    xtR = np.ascontiguousarray(xb.reshape(NT, 512, KH, 128).transpose(0, 3, 2, 1))

    w_gate = inputs["w_gate"]
    w_up = inputs["w_up"]
    w_down = inputs["w_down"]
    ws_gate = inputs["ws_gate"].astype(BF16)
    ws_up = inputs["ws_up"].astype(BF16)
    ws_down = inputs["ws_down"].astype(BF16)

    Csum = sum(caps)
    in_maps = []
    for c in range(N_CORES):
        wtsR = np.zeros((128, Csum), np.float32)
        wgR = np.empty((EPC, NF, 128, KH, 128), F8)
        wuR = np.empty((EPC, NF, 128, KH, 128), F8)
        wdR = np.empty((EPC, NH, 128, NF, 512), F8)
        imap = {}
        for el in range(EPC):
            C = caps[el]
            ge = int(slot_expert[c, el])
            rows, wts = rows_per_e[ge]
            n = len(rows)
            xgR = np.zeros((128, KH, C), F8)
            if n:
                # [n, H] -> [128, KH, n]
                xgR[:, :, :n] = x8[rows].reshape(n, KH, 128).transpose(2, 1, 0)
                base = sum(caps[:el])
                # fold the fp8 dequant (w_down 64x, w_up 4x) into the
                # per-token routing weights applied at PSUM eviction,
                # broadcast across all 128 partitions (H columns)
                wtsR[:, base:base + n] = wts / (S_W * S_WU)
            imap[f"xg{el}"] = xgR
            wgR[el] = (
                (w_gate[ge] * S_W).astype(F8)
                .reshape(KH, 128, NF, 128).transpose(2, 1, 0, 3)
            )
            wuR[el] = (
                (w_up[ge] * S_WU).astype(F8)
                .reshape(KH, 128, NF, 128).transpose(2, 1, 0, 3)
            )
            wdR[el] = (
                (w_down[ge] * S_W).astype(F8)
                .reshape(NF, 128, NH, 512).transpose(2, 1, 0, 3)
            )
        sl = slice(c * FSS, (c + 1) * FSS)
        wsgp = np.zeros((H, FSP), BF16)
        wsgp[:, :FSS] = ws_gate[:, sl]
        wsup = np.zeros((H, FSP), BF16)
        wsup[:, :FSS] = ws_up[:, sl]
        wsdp = np.zeros((FSP, H), BF16)
        wsdp[:FSS] = ws_down[sl]
        wsg3 = np.empty((NFS, 128, KH, 128), BF16)
        wsu3 = np.empty((NFS, 128, KH, 128), BF16)
        wsd3 = np.empty((NFS, 128, NH, 512), BF16)
        for q in range(NFS):
            cols = slice(q * 128, (q + 1) * 128)
            wsg3[q] = wsgp[:, cols].reshape(KH, 128, 128).transpose(1, 0, 2)
            wsu3[q] = wsup[:, cols].reshape(KH, 128, 128).transpose(1, 0, 2)
            wsd3[q] = wsdp[cols].reshape(128, NH, 512)
        imap.update(
            wg=wgR,
            wu=wuR,
            wd=wdR,
            xt=xtR,
            wsg=wsg3,
            wsu=wsu3,
            wsd=wsd3,
            wts=wtsR,
        )
        in_maps.append(imap)

    res = run_bass_kernel_spmd(nc, in_maps, core_ids=list(range(N_CORES)))
    last_exec_time_ns = res.exec_time_ns

    out = np.zeros((T, H), np.float32)
    for c in range(N_CORES):
        r = res.results[c]
        # sout blocks [NT, 4, 128, NH, 512] -> [T, H] (already token-major)
        out += r["sout"].astype(np.float32).reshape(T, H)
        for el in range(EPC):
            rows, _ = rows_per_e[int(slot_expert[c, el])]
            n = len(rows)
            if n:
                # rout blocks [NH, nch, 128, 4, cw] -> [H, C]; rows are
                # unique within one expert, so fancy-index add is safe
                re_ = (
                    r[f"rout{el}"].astype(np.float32)
                    .transpose(0, 3, 2, 1, 4).reshape(H, caps[el])
                )
                out[rows] += re_[:, :n].T
    return out.reshape(hs.shape).astype(hs.dtype)


# revision 39
# speedup vs baseline: 1.0335x; 1.0335x over previous
"""Trainium2 Bass kernel for a DeepSeek-style MoE block (full-I/O contract).

Strategy (8 NeuronCores):
  - Expert-parallel: E=16 routed experts, 2 per core. Host computes the gate
    (softmax + top-4) in numpy, gathers each expert's tokens, and ships
    transposed token blocks per core. Experts are ranked by token count:
    ranks 0-7 go to slot 0 (capacity C0), ranks 8-15 to slot 1 (C1 <= C0),
    so padding waste tracks the actual load distribution.
  - Routed path runs in fp8 e4m3 with DoubleRow matmuls (256-deep
    contraction per instruction, 2x PE throughput). Weights are scaled
    into fp8 range on the host (w_gate/w_down by 64, w_up by 4); the
    gate dequant rides the SiLU activation's input scale and the rest is
    folded into the per-token routing weights at PSUM eviction. The
    shared expert stays in bf16: it carries ~98% of the output norm,
    while the routed path (~22%) tolerates fp8 noise.
  - Shared expert is tensor-parallel along its intermediate dim Fs=2816:
    each core owns a 352-wide slice (zero-padded to 384 = 3*128).
  - Host scatter-adds the routed partials and sums the shared partials.
"""

import math
from contextlib import ExitStack

import ml_dtypes
import numpy as np

T = 2048
H = 2048
E = 16
TOP_K = 4
F = 1408
FS = 2816
N_CORES = 8
EPC = E // N_CORES  # experts per core = 2
KH = H // 128  # 16 contraction chunks over H
KH2 = KH // 2  # 8 DoubleRow pairs
NF = F // 128  # 11 F tiles
NF2 = NF // 2  # 5 DoubleRow pairs (+1 leftover chunk)
NH = H // 512  # 4 output H tiles
FSS = FS // N_CORES  # 352 shared slice per core
FSP = 384  # padded to 3*128
NFS = FSP // 128  # 3
NT = T // 512  # 4

S_W = 64.0  # fp8 scale for w_gate / w_down
S_WU = 4.0  # fp8 scale for w_up (rides through silu(g)*u into the a tiles)

BF16 = ml_dtypes.bfloat16
F8 = ml_dtypes.float8_e4m3

_BUILD_CACHE: dict[tuple, object] = {}
last_exec_time_ns = None


def _routing(x: np.ndarray, gate_weight: np.ndarray):
    """Replicates the reference gate: fp32 logits, softmax, top-4 (ties ->
    lower expert index, matching jax.lax.top_k), no renorm."""
    logits = x.astype(np.float32) @ gate_weight.astype(np.float32).T
    z = logits - logits.max(axis=1, keepdims=True)
    p = np.exp(z)
    p /= p.sum(axis=1, keepdims=True)
    top_idx = np.argsort(-p, axis=1, kind="stable")[:, :TOP_K]
    top_vals = np.take_along_axis(p, top_idx, axis=1).astype(np.float32)
    return top_idx, top_vals


def _chunks(C):
    n = max(1, math.ceil(C / 512))
    while C % n:
        n += 1
    return C // n


def _build(caps: tuple):
    """Build + compile the SPMD one-core Bass graph for slot capacities."""
    key = tuple(caps)
    if key in _BUILD_CACHE:
        return _BUILD_CACHE[key]

    import concourse.bass as bass  # noqa: F401
    from concourse import bacc, mybir, tile

    bf = mybir.dt.bfloat16
    f8 = mybir.dt.float8e4
    f32 = mybir.dt.float32
    Silu = mybir.ActivationFunctionType.Silu
    DR = mybir.MatmulPerfMode.DoubleRow

    off = [0, caps[0]]  # column offsets into rout / wts

    nc = bacc.Bacc(None, target_bir_lowering=False)

    xg_ds = [
        nc.dram_tensor(f"xg{e}", [128, KH, caps[e]], f8, kind="ExternalInput")
        for e in range(EPC)
    ]
    wg_d = nc.dram_tensor("wg", [EPC, NF, 128, KH, 128], f8, kind="ExternalInput")
    wu_d = nc.dram_tensor("wu", [EPC, NF, 128, KH, 128], f8, kind="ExternalInput")
    wd_d = nc.dram_tensor("wd", [EPC, NH, 128, NF, 512], f8, kind="ExternalInput")
    xt_d = nc.dram_tensor("xt", [NT, 128, KH, 512], bf, kind="ExternalInput")
    wsg_d = nc.dram_tensor("wsg", [NFS, 128, KH, 128], bf, kind="ExternalInput")
    wsu_d = nc.dram_tensor("wsu", [NFS, 128, KH, 128], bf, kind="ExternalInput")
    wsd_d = nc.dram_tensor("wsd", [NFS, 128, NH, 512], bf, kind="ExternalInput")
    wts_d = nc.dram_tensor("wts", [128, sum(caps)], f32, kind="ExternalInput")
    # outputs are block-contiguous per eviction tile (one DMA descriptor
    # each); the host reassembles token/H-major layouts
    rout_ds = [
        nc.dram_tensor(
            f"rout{e}", [NH, caps[e] // _chunks(caps[e]), 128, 4, _chunks(caps[e])],
            bf, kind="ExternalOutput",
        )
        for e in range(EPC)
    ]
    sout = nc.dram_tensor("sout", [NT, 4, 128, NH, 512], bf, kind="ExternalOutput")

    with tile.TileContext(nc) as tc, ExitStack() as ctx:
        const = ctx.enter_context(tc.tile_pool(name="const", bufs=1))
        bias0 = const.tile([128, 1], f32)
        nc.vector.memset(bias0[:], 0.0)
        # routing weights broadcast across partitions: wts_t[p, col] = w[col]
        wts_t = const.tile([128, sum(caps)], f32)
        nc.gpsimd.dma_start(wts_t[:], wts_d[:])


        xt_pool = ctx.enter_context(tc.tile_pool(name="xtp", bufs=4))
        wgu_pool = ctx.enter_context(tc.tile_pool(name="wgu", bufs=4))
        as_pool = ctx.enter_context(tc.tile_pool(name="asp", bufs=2))
        wsd_pool = ctx.enter_context(tc.tile_pool(name="wsdp", bufs=1))
        ev_pool = ctx.enter_context(tc.tile_pool(name="evp", bufs=4))
        sg_pool = ctx.enter_context(tc.tile_pool(name="sgp", bufs=4))
        psum_gu = ctx.enter_context(tc.tile_pool(name="pgu", bufs=2, space="PSUM"))
        psum_d = ctx.enter_context(tc.tile_pool(name="pdp", bufs=4, space="PSUM"))

        # HAM warmup: keep the PE busy during the initial DMA wait so the
        # clock-gate is at 8/8 when the first real matmuls arrive
        warm = const.tile([128, 512], bf, name="warm")
        nc.vector.memset(warm[:], 0.0)
        warmout = const.tile([128, 1], f32, name="warmout")
        wpsum = psum_gu.tile([128, 512], f32, tag="pg", name="warmp")
        for _ in range(30):
            nc.tensor.matmul(wpsum, warm[:, :128], warm[:], start=True, stop=True)
        nc.vector.tensor_copy(out=warmout[:], in_=wpsum[:, :1])

        # ---- shared expert (Fs tensor-parallel slice, bf16) ----
        wsd_ts = []
        for j in range(NT):
            xt_t = xt_pool.tile([128, KH, 512], bf, tag="xt", name="xt")
            for q in range(4):
                eng = nc.gpsimd if j == 0 and q < 2 else nc.sync
                eng.dma_start(
                    xt_t[:, q * 4:(q + 1) * 4], xt_d[j, :, q * 4:(q + 1) * 4]
                )
            as_j = as_pool.tile([128, NFS, 512], bf, tag="asj", name="asj")
            for f in range(NFS):
                wsg_t = wgu_pool.tile([128, KH, 128], bf, tag="wg")
                wsu_t = wgu_pool.tile([128, KH, 128], bf, tag="wu")
                if j == 0 and f == 0:
                    # startup: spread the critical first loads over two
                    # DMA queues (SP / Activation run in parallel)
                    nc.sync.dma_start(wsg_t[:, :8], wsg_d[f, :, :8])
                    nc.sync.dma_start(wsg_t[:, 8:], wsg_d[f, :, 8:])
                    nc.scalar.dma_start(wsu_t[:, :8], wsu_d[f, :, :8])
                    nc.scalar.dma_start(wsu_t[:, 8:], wsu_d[f, :, 8:])
                else:
                    nc.sync.dma_start(wsg_t[:, :8], wsg_d[f, :, :8])
                    nc.sync.dma_start(wsg_t[:, 8:], wsg_d[f, :, 8:])
                    nc.sync.dma_start(wsu_t[:, :8], wsu_d[f, :, :8])
                    nc.sync.dma_start(wsu_t[:, 8:], wsu_d[f, :, 8:])
                pg = psum_gu.tile([128, 512], f32, tag="pg", name="pg")
                pu = psum_gu.tile([128, 512], f32, tag="pu", name="pu")
                for k in range(KH):
                    nc.tensor.matmul(
                        pg, wsg_t[:, k], xt_t[:, k],
                        start=(k == 0), stop=(k == KH - 1),
                    )
                for k in range(KH):
                    nc.tensor.matmul(
                        pu, wsu_t[:, k], xt_t[:, k],
                        start=(k == 0), stop=(k == KH - 1),
                    )
                sg = sg_pool.tile([128, 512], f32, tag="sg", name="sg")
                nc.scalar.activation(sg, pg, Silu, bias=bias0[:])
                nc.vector.tensor_mul(as_j[:, f], sg, pu)
            if j == 1:
                gate_src = as_j
            if not wsd_ts:
                for cc in range(NFS):
                    w = wsd_pool.tile(
                        [128, NH, 512], bf, tag=f"wsd{cc}", name=f"wsd{cc}"
                    )
                    nc.sync.dma_start(w[:, :2], wsd_d[cc, :, :2])
                    nc.sync.dma_start(w[:, 2:], wsd_d[cc, :, 2:])
                    wsd_ts.append(w)
            # down-proj for this T-chunk right away: spreads sout writes
            for tl in range(4):
                ob = ev_pool.tile([128, NH, 512], bf, tag="ob")
                for h in range(NH):
                    pd = psum_d.tile([128, 512], f32, tag="pd")
                    for fo in range(NFS):
                        nc.tensor.matmul(
                            pd, as_j[:, fo, tl * 128:(tl + 1) * 128],
                            wsd_ts[fo][:, h],
                            start=(fo == 0), stop=(fo == NFS - 1),
                        )
                    nc.vector.tensor_copy(out=ob[:, h], in_=pd[:])
                nc.scalar.dma_start(sout[j, tl], ob[:])

        # ---- routed experts (2 per core, fp8 DoubleRow, C0 >= C1) ----
        with (
            tc.tile_pool(name="xgp", bufs=1) as xg_pool,
            tc.tile_pool(name="wdp", bufs=2) as wd_pool,
            tc.tile_pool(name="atp", bufs=1) as a_pool,
        ):
            for e in range(EPC):
                C = caps[e]
                cw = _chunks(C)
                nch = C // cw
                xg_t = xg_pool.tile([128, KH, C], f8, tag=f"xg{e}", name=f"xg{e}")
                # WAW anchor: the tiny write forces the bulk load to wait
                # for shared-phase progress, keeping early DMA bandwidth
                # on the shared expert's critical loads
                nc.gpsimd.tensor_copy(out=xg_t[:1, 0, :1], in_=gate_src[:1, 0, :1])
                for q in range(4):
                    nc.gpsimd.dma_start(
                        xg_t[:, q * 4:(q + 1) * 4], xg_ds[e][:, q * 4:(q + 1) * 4]
                    )
                aT = a_pool.tile([128, NF, C], f8, tag=f"aT{e}", name=f"aT{e}")
                for f in range(NF):
                    wg_t = wgu_pool.tile([128, KH, 128], f8, tag="wg")
                    nc.gpsimd.dma_start(wg_t[:], wg_d[e, f])
                    wu_t = wgu_pool.tile([128, KH, 128], f8, tag="wu")
                    nc.scalar.dma_start(wu_t[:], wu_d[e, f])
                    for j in range(nch):
                        csl = slice(j * cw, (j + 1) * cw)
                        pg = psum_gu.tile([128, 512], f32, tag="pg", name="pg")[:, :cw]
                        pu = psum_gu.tile([128, 512], f32, tag="pu", name="pu")[:, :cw]
                        for k in range(KH2):
                            nc.tensor.matmul(
                                pg, wg_t[:, 2 * k:2 * k + 2],
                                xg_t[:, 2 * k:2 * k + 2, csl],
                                start=(k == 0), stop=(k == KH2 - 1),
                                perf_mode=DR,
                            )
                        for k in range(KH2):
                            nc.tensor.matmul(
                                pu, wu_t[:, 2 * k:2 * k + 2],
                                xg_t[:, 2 * k:2 * k + 2, csl],
                                start=(k == 0), stop=(k == KH2 - 1),
                                perf_mode=DR,
                            )
                        sg = sg_pool.tile([128, 512], f32, tag="sg", name="sg")[:, :cw]
                        # dequant w_gate's 64x scale ahead of the nonlinearity
                        nc.scalar.activation(
                            sg, pg, Silu, bias=bias0[:], scale=1.0 / S_W
                        )
                        # a tile keeps w_up's 4x scale; folded into wts below
                        nc.vector.tensor_mul(aT[:, f, csl], sg, pu)
                # down-proj with w_down stationary and token columns moving:
                # cost scales with C, not ceil(C/128)*512
                for h in range(NH):
                    wd_t = wd_pool.tile([128, NF, 512], f8, tag="wd")
                    if e == 0 and h < 2:
                        nc.gpsimd.tensor_copy(
                            out=wd_t[:1, 0, :1], in_=gate_src[:1, 0, :1]
                        )
                    nc.gpsimd.dma_start(wd_t[:, :6], wd_d[e, h, :, :6])
                    nc.gpsimd.dma_start(wd_t[:, 6:], wd_d[e, h, :, 6:])
                    for j in range(nch):
                        csl = slice(j * cw, (j + 1) * cw)
                        ob = ev_pool.tile([128, 4, 512], bf, tag="ob", name="ob")[
                            :, :, :cw
                        ]
                        for hs in range(4):
                            hsl = slice(hs * 128, (hs + 1) * 128)
                            pd = psum_d.tile([128, 512], f32, tag="pd", name="pd")[
                                :, :cw
                            ]
                            for f2 in range(NF2):
                                nc.tensor.matmul(
                                    pd, wd_t[:, 2 * f2:2 * f2 + 2, hsl],
                                    aT[:, 2 * f2:2 * f2 + 2, csl],
                                    start=(f2 == 0), stop=False,
                                    perf_mode=DR,
                                )
                            nc.tensor.matmul(
                                pd, wd_t[:, NF - 1, hsl], aT[:, NF - 1, csl],
                                start=False, stop=True,
                            )
                            nc.vector.tensor_mul(
                                ob[:, hs], pd[:], wts_t[:, off[e] + j * cw:
                                                        off[e] + j * cw + cw]
                            )
                        if e == EPC - 1 and h >= NH - 2:
                            # kernel tail: drain the last evictions in
                            # partition-quarters on all three DMA queues
                            engs = [nc.scalar, nc.sync, nc.gpsimd, nc.sync]
                            for pq in range(4):
                                psl = slice(pq * 32, (pq + 1) * 32)
                                engs[pq].dma_start(
                                    rout_ds[e][h, j, psl], ob[psl]
                                )
                        else:
                            eng = nc.sync if (h * nch + j) % 2 else nc.scalar
                            eng.dma_start(rout_ds[e][h, j], ob[:])

    nc.compile()
    _BUILD_CACHE[key] = nc
    return nc


def kernel(**inputs: np.ndarray) -> np.ndarray:
    global last_exec_time_ns
    from concourse.bass_utils import run_bass_kernel_spmd

    hs = inputs["hidden_states"]
    x = np.ascontiguousarray(hs.reshape(T, H), dtype=np.float32)
    top_idx, top_vals = _routing(x, inputs["gate_weight"])

    # per-expert token lists (ascending token order)
    rows_per_e = []
    for e in range(E):
        rows, kpos = np.nonzero(top_idx == e)
        rows_per_e.append((rows, top_vals[rows, kpos]))
    counts = np.array([len(r) for r, _ in rows_per_e])
    # rank experts by load: ranks 0..7 -> slot 0 of cores 0..7 (big slots),
    # ranks 8..15 -> slot 1 of cores 7..0 (small slots)
    order = np.argsort(-counts, kind="stable")
    slot_expert = np.empty((N_CORES, EPC), np.int64)
    for i in range(N_CORES):
        slot_expert[i, 0] = order[i]
        slot_expert[i, 1] = order[E - 1 - i]
    cap = lambda n: max(128, ((n + 15) // 16) * 16)
    caps = (
        cap(int(counts[slot_expert[:, 0]].max())),
        cap(int(counts[slot_expert[:, 1]].max())),
    )

    nc = _build(caps)

    xb = x.astype(BF16)
    x8 = x.astype(F8)
    # xt chunks [NT, 128, KH, 512]: xt[j, p, k, t'] = x[j*512+t', k*128+p]
    xtR = np.ascontiguousarray(xb.reshape(NT, 512, KH, 128).transpose(0, 3, 2, 1))

    w_gate = inputs["w_gate"]
    w_up = inputs["w_up"]
    w_down = inputs["w_down"]
    ws_gate = inputs["ws_gate"].astype(BF16)
    ws_up = inputs["ws_up"].astype(BF16)
    ws_down = inputs["ws_down"].astype(BF16)

    Csum = sum(caps)
    in_maps = []
    for c in range(N_CORES):
        wtsR = np.zeros((128, Csum), np.float32)
        wgR = np.empty((EPC, NF, 128, KH, 128), F8)
        wuR = np.empty((EPC, NF, 128, KH, 128), F8)
        wdR = np.empty((EPC, NH, 128, NF, 512), F8)
        imap = {}
        for el in range(EPC):
            C = caps[el]
            ge = int(slot_expert[c, el])
            rows, wts = rows_per_e[ge]
            n = len(rows)
            xgR = np.zeros((128, KH, C), F8)
            if n:
                # [n, H] -> [128, KH, n]
                xgR[:, :, :n] = x8[rows].reshape(n, KH, 128).transpose(2, 1, 0)
                base = sum(caps[:el])
                # fold the fp8 dequant (w_down 64x, w_up 4x) into the
                # per-token routing weights applied at PSUM eviction,
                # broadcast across all 128 partitions (H columns)
                wtsR[:, base:base + n] = wts / (S_W * S_WU)
            imap[f"xg{el}"] = xgR
            wgR[el] = (
                (w_gate[ge] * S_W).astype(F8)
                .reshape(KH, 128, NF, 128).transpose(2, 1, 0, 3)
            )
            wuR[el] = (
                (w_up[ge] * S_WU).astype(F8)
                .reshape(KH, 128, NF, 128).transpose(2, 1, 0, 3)
            )
            wdR[el] = (
                (w_down[ge] * S_W).astype(F8)
                .reshape(NF, 128, NH, 512).transpose(2, 1, 0, 3)
            )
        sl = slice(c * FSS, (c + 1) * FSS)
        wsgp = np.zeros((H, FSP), BF16)
        wsgp[:, :FSS] = ws_gate[:, sl]
        wsup = np.zeros((H, FSP), BF16)
        wsup[:, :FSS] = ws_up[:, sl]
        wsdp = np.zeros((FSP, H), BF16)
        wsdp[:FSS] = ws_down[sl]
        wsg3 = np.empty((NFS, 128, KH, 128), BF16)
        wsu3 = np.empty((NFS, 128, KH, 128), BF16)
        wsd3 = np.empty((NFS, 128, NH, 512), BF16)
        for q in range(NFS):
            cols = slice(q * 128, (q + 1) * 128)
            wsg3[q] = wsgp[:, cols].reshape(KH, 128, 128).transpose(1, 0, 2)
            wsu3[q] = wsup[:, cols].reshape(KH, 128, 128).transpose(1, 0, 2)
            wsd3[q] = wsdp[cols].reshape(128, NH, 512)
        imap.update(
            wg=wgR,
            wu=wuR,
            wd=wdR,
            xt=xtR,
            wsg=wsg3,
            wsu=wsu3,
            wsd=wsd3,
            wts=wtsR,
        )
        in_maps.append(imap)

    res = run_bass_kernel_spmd(nc, in_maps, core_ids=list(range(N_CORES)))
    last_exec_time_ns = res.exec_time_ns

    out = np.zeros((T, H), np.float32)
    for c in range(N_CORES):
        r = res.results[c]
        # sout blocks [NT, 4, 128, NH, 512] -> [T, H] (already token-major)
        out += r["sout"].astype(np.float32).reshape(T, H)
        for el in range(EPC):
            rows, _ = rows_per_e[int(slot_expert[c, el])]
            n = len(rows)
            if n:
                # rout blocks [NH, nch, 128, 4, cw] -> [H, C]; rows are
                # unique within one expert, so fancy-index add is safe
                re_ = (
                    r[f"rout{el}"].astype(np.float32)
                    .transpose(0, 3, 2, 1, 4).reshape(H, caps[el])
                )
                out[rows] += re_[:, :n].T
    return out.reshape(hs.shape).astype(hs.dtype)
